# revision 2
# baseline (speedup 1.0000x reference)
"""Distributed TRN2 Bass kernel for a dynamic-int8-quantized transformer encoder layer.

Math (matches the jax reference bit-closely):
  q = fq(x) @ fq(Wq).T + bq          (per-tensor dynamic symmetric int8 fake-quant)
  k, v likewise; attn = softmax(fq(q/sqrt(d)) @ fq(k).T); out = fq(attn) @ fq(v)
  final = fq(out) @ fq(Wp).T + bp

Implementation:
  - int8 fake-quant matmuls computed exactly as integer-valued bf16 matmuls
    (integers <= 127 exact in bf16, fp32 PSUM accumulation), dequant scales on
    PSUM eviction.
  - Sharding: tokens (B*S = 4096) split 8 ways; cores 0-3 own batch 0, cores 4-7
    batch 1. Quantized K^T/V AllGather'd within each 4-core batch group; weight
    quantization sharded 8 ways and AllGather'd; global per-tensor amaxes via
    tiny AllReduce-max collectives.
  - softmax without max-subtraction (|logits| << 88): probs = exp(a)/S_i.
    Global amax(probs) = max_i M_i/S_i from fused per-row stats. The quantized
    transposed probs for S@V come from recomputing QK^T with an extra k=2
    matmul adding c'_i = -ln(S_i*s_attn)/s_qk (bf16 hi+lo split), so
    round(exp(s_qk*(a_int + c'_i))) yields the int8 prob directly.
  - round-to-nearest-even: x + 1.5*2^23 - 1.5*2^23 in fp32 (matches jnp.round).
"""

import math
import os
import sys
from contextlib import ExitStack

import numpy as np

sys.path.insert(0, "/opt/trn_rl_repo")

B = 2
S = 2048
H = 2048
NH = 16
D = 128
NC = 8
GROUP = 4          # cores per batch
TLOC = 512         # tokens per core
OSL = H // NC      # 256: per-core slice of W output dims
Q_MUL = 1.0 / math.sqrt(D)
RMAGIC = 12582912.0  # 1.5 * 2**23

_COMPILED = {}


def _build(debug=False):
    import concourse.mybir as mybir
    import concourse.tile as tile
    from concourse import bacc
    from concourse import bass_isa

    f32 = mybir.dt.float32
    bf16 = mybir.dt.bfloat16
    AF = mybir.ActivationFunctionType
    OP = mybir.AluOpType
    AX = mybir.AxisListType

    nc = bacc.Bacc(None, target_bir_lowering=False, debug=False, num_devices=NC)

    x_T = nc.declare_dram_parameter("x_T", [H, TLOC], f32, isOutput=False)
    wsl = nc.declare_dram_parameter("wsl", [4, H, OSL], f32, isOutput=False)
    bq_t = nc.declare_dram_parameter("bq_t", [128, 16], f32, isOutput=False)
    bk_t = nc.declare_dram_parameter("bk_t", [128, 16], f32, isOutput=False)
    bp_t = nc.declare_dram_parameter("bp_t", [128, 16], f32, isOutput=False)
    bv_b = nc.declare_dram_parameter("bv_b", [128, H], f32, isOutput=False)
    out_ext = nc.declare_dram_parameter("out", [H, TLOC], f32, isOutput=True)
    dbg = {}
    if debug:
        for name, shape in [
            ("d_qdeq", [128, 16, TLOC]), ("d_kdeq", [128, 16, TLOC]),
            ("d_vdeq", [128, 4, H]), ("d_scales", [1, 16]),
            ("d_S", [128, 64]), ("d_M", [128, 64]),
            ("d_pint", [128, 16, TLOC]), ("d_outT", [128, 16, TLOC]),
        ]:
            dbg[name] = nc.declare_dram_parameter(name, shape, f32, isOutput=True)

    grp = [list(range(0, GROUP)), list(range(GROUP, NC))]
    allg = [list(range(NC))]

    with tile.TileContext(nc) as tc, ExitStack() as top:
        dram = top.enter_context(tc.tile_pool(name="dram", bufs=1, space="DRAM"))
        # Wq/Wk/Wp: stationary-tiled [half][p][ht][o'] (4KB read rows);
        # Wv: row-major [ht][p][o'] (512B moving-read rows)
        w_bounces = [
            dram.tile(
                [2, 128, 16, 128] if w != 2 else [16, 128, OSL],
                bf16, name=f"wb{w}",
            )
            for w in range(4)
        ]
        wgs = [
            dram.tile(
                [NC, 2, 128, 16, 128] if w != 2 else [NC, 16, 128, OSL],
                bf16, addr_space="Shared", name=f"wg{w}",
            )
            for w in range(4)
        ]
        k_bs = [dram.tile([H * TLOC // 2], bf16, name=f"kb{i}") for i in range(2)]
        v_bounce = dram.tile([H * TLOC], bf16)
        k_gs = [dram.tile([GROUP, H * TLOC // 2], bf16, name=f"kg{i}") for i in range(2)]
        v_g = dram.tile([GROUP, H * TLOC], bf16)
        ar1_i = dram.tile([1, 5], f32)
        ar1a_o = dram.tile([1, 2], f32, addr_space="Shared")
        ar1b_o = dram.tile([1, 3], f32, addr_space="Shared")
        ar2_i = dram.tile([1, 3], f32)
        ar2_o = dram.tile([1, 3], f32, addr_space="Shared")
        ar3_i = dram.tile([1, 1], f32)
        ar3_o = dram.tile([1, 1], f32, addr_space="Shared")
        ar4_i = dram.tile([1, 1], f32)
        ar4_o = dram.tile([1, 1], f32, addr_space="Shared")
        rt_bf = dram.tile([3, 16, 512], bf16)  # c' hi/mid/lo rows, [h][it*128+p]

        const = top.enter_context(tc.tile_pool(name="const", bufs=1))
        sc = top.enter_context(tc.tile_pool(name="scal", bufs=1))
        sbuf = top.enter_context(tc.tile_pool(name="sbuf_main", bufs=1))
        qpool = top.enter_context(tc.tile_pool(name="qscratch", bufs=3))

        ones3 = const.tile([3, 128], bf16)
        nc.vector.memset(ones3[:], 1.0)

        # All scalars/broadcasts live as columns of one tile; partials in another.
        scal = sc.tile([128, 96], f32, name="scal")
        _col = [0]

        def cols(n):
            c0 = _col[0]
            _col[0] += n
            assert _col[0] <= 96
            return scal[:, c0 : c0 + n]

        parts = sc.tile([128, 192], f32, name="parts")
        _pcol = [0]

        def pcols(n):
            c0 = _pcol[0]
            _pcol[0] += n
            assert _pcol[0] <= 192
            return parts[:, c0 : c0 + n]

        def p_reduce_max(part_col):
            """(128,1) fp32 -> (128,1) all-partitions-equal global max."""
            red = cols(1)
            nc.gpsimd.partition_all_reduce(
                red, part_col, channels=128, reduce_op=bass_isa.ReduceOp.max
            )
            return red

        def bcast(src1n):
            """(1,n) fp32 -> (128,n)."""
            b = cols(src1n.shape[-1])
            nc.gpsimd.partition_broadcast(b, src1n)
            return b

        def rnd(out_ap, in_ap):
            nc.vector.tensor_scalar(
                out_ap, in_ap, RMAGIC, RMAGIC, op0=OP.add, op1=OP.subtract
            )

        # long-lived singles, allocated in stack (LIFO) order
        q_int, q_int_free = tc.tile([128, 16, TLOC], bf16, name="q_int")
        x_int, x_int_free = tc.tile([128, 16, TLOC], bf16, name="x_int")

        # ============ P0/P1: load x; amax of x and W slices; AR1 ============
        x_f32, x_f32_free = tc.tile([128, 16, TLOC], f32, name="x_f32")
        nc.sync.dma_start(out=x_f32[:], in_=x_T.rearrange("(ht p) t -> p ht t", p=128))
        xparts = pcols(16)
        for ht in range(16):
            nc.vector.tensor_reduce(
                xparts[:, ht : ht + 1], x_f32[:, ht, :], AX.X, OP.max,
                apply_absolute_value=True,
            )
        xa = pcols(1)
        nc.vector.tensor_reduce(xa, xparts, AX.X, OP.max)
        xag = p_reduce_max(xa)
        nc.sync.dma_start(out=ar1_i[:, 0:1], in_=xag[0:1, :])

        wsl_r = wsl.rearrange("w (ht p) o -> w p ht o", p=128)
        g5 = cols(5)[0:1, :]
        s5 = cols(5)[0:1, :]
        i5 = cols(5)[0:1, :]
        i5b = cols(5)
        sxw = cols(3)[0:1, :]
        sxwb = cols(3)

        def w_amax(w, wf_pool, slot):
            wap = pcols(16)
            for ht in range(16):
                wt = wf_pool.tile([128, OSL], f32, name="wt")
                nc.sync.dma_start(out=wt[:], in_=wsl_r[w, :, ht, :])
                nc.vector.tensor_reduce(
                    wap[:, ht : ht + 1], wt[:], AX.X, OP.max,
                    apply_absolute_value=True,
                )
            wa = pcols(1)
            nc.vector.tensor_reduce(wa, wap, AX.X, OP.max)
            wag = p_reduce_max(wa)
            nc.sync.dma_start(out=ar1_i[:, slot : slot + 1], in_=wag[0:1, :])

        def scales_from(sl, src_t):
            nc.sync.dma_start(out=g5[:, sl], in_=src_t[:])
            nc.vector.tensor_scalar(
                s5[:, sl], g5[:, sl], 1.0 / 127.0, 1e-8, op0=OP.mult, op1=OP.max
            )
            nc.vector.reciprocal(i5[:, sl], s5[:, sl])
            nc.gpsimd.partition_broadcast(i5b[:, sl], i5[:, sl])

        with tc.tile_pool(name="wf", bufs=4) as wf_pool:
            # ---- AR1a: x + Wq ----
            w_amax(0, wf_pool, 1)
            nc.gpsimd.collective_compute(
                "AllReduce", OP.max, replica_groups=allg,
                ins=[ar1_i[:, 0:2].opt()], outs=[ar1a_o[:].opt()],
            )
            scales_from(slice(0, 2), ar1a_o)
            nc.vector.tensor_mul(sxw[:, 0:1], s5[:, 0:1], s5[:, 1:2])
            nc.gpsimd.partition_broadcast(sxwb[:, 0:1], sxw[:, 0:1])

            # quantize x immediately (gates all projections)
            for ht in range(16):
                xm = qpool.tile([128, TLOC], f32, name="xm", tag="qs_f32")
                nc.scalar.activation(xm[:], x_f32[:, ht, :], AF.Copy, scale=i5b[:, 0:1])
                rnd(x_int[:, ht, :], xm[:])

            # ---- AR1b: Wk, Wv, Wp ----
            for w in (1, 2, 3):
                w_amax(w, wf_pool, 1 + w)
            nc.gpsimd.collective_compute(
                "AllReduce", OP.max, replica_groups=allg,
                ins=[ar1_i[:, 2:5].opt()], outs=[ar1b_o[:].opt()],
            )
            scales_from(slice(2, 5), ar1b_o)
            for w in (1, 2):
                nc.vector.tensor_mul(sxw[:, w : w + 1], s5[:, 0:1], s5[:, 1 + w : 2 + w])
            nc.gpsimd.partition_broadcast(sxwb[:, 1:3], sxw[:, 1:3])

            # ============ P2: quantize W slices; per-W AllGather ============
            for w in range(4):
                if w != 2:
                    wb_r = w_bounces[w].rearrange("half p ht o -> p half ht o")
                else:
                    wb_r = w_bounces[w].rearrange("ht p o -> p ht o")
                for ht in range(16):
                    wt = wf_pool.tile([128, OSL], f32, name="wt2")
                    nc.sync.dma_start(out=wt[:], in_=wsl_r[w, :, ht, :])
                    wm = qpool.tile([128, OSL], f32, name="wm", tag="qs_f32")
                    nc.scalar.activation(wm[:], wt[:], AF.Copy, scale=i5b[:, 1 + w : 2 + w])
                    wi = qpool.tile([128, OSL], bf16, name="wi", tag="qs_bf16")
                    rnd(wi[:], wm[:])
                    if w != 2:
                        nc.sync.dma_start(
                            out=wb_r[:, :, ht, :],
                            in_=wi[:].rearrange("p (half o) -> p half o", o=128),
                        )
                    else:
                        nc.sync.dma_start(out=wb_r[:, ht, :], in_=wi[:])
                nc.gpsimd.collective_compute(
                    "AllGather", OP.bypass, replica_groups=allg,
                    ins=[w_bounces[w][:].opt()], outs=[wgs[w][:].opt()],
                )
        x_f32_free()

        # stationary tiles for w in {0,1,3}: wgs[w][r, half] -> (128 p, 16 ht, 128 o)
        def w_col_ap(w, ot):
            assert w != 2
            return wgs[w][ot // 2, ot % 2]

        # ============ P3: QKV projections ============
        bq_sb = const.tile([128, 16], f32)
        nc.sync.dma_start(out=bq_sb[:], in_=bq_t[:, :])
        bk_sb = const.tile([128, 16], f32)
        nc.sync.dma_start(out=bk_sb[:], in_=bk_t[:, :])
        bp_sb = const.tile([128, 16], f32)
        nc.sync.dma_start(out=bp_sb[:], in_=bp_t[:, :])
        bv_sb = const.tile([128, H], f32)
        nc.sync.dma_start(out=bv_sb[:], in_=bv_b[:, :])

        q_deq, q_deq_free = tc.tile([128, 16, TLOC], f32, name="q_deq")
        k_deq, k_deq_free = tc.tile([128, 16, TLOC], f32, name="k_deq")
        v_deq, v_deq_free = tc.tile([128, 4, H], f32, name="v_deq")
        qa_parts = pcols(16)
        ka_parts = pcols(16)
        va_parts = pcols(4)

        with tc.tile_pool(name="qkv_psum", bufs=4, space="PSUM") as qkv_psum, \
             tc.tile_pool(name="wcol", bufs=3) as wcol_pool, \
             tc.tile_pool(name="wvm", bufs=18) as wvm_pool:
            for dst, wi_, bias_sb, scol, aparts in (
                (q_deq, 0, bq_sb, 0, qa_parts),
                (k_deq, 1, bk_sb, 1, ka_parts),
            ):
                for ot in range(16):
                    wcol = wcol_pool.tile([128, 16, 128], bf16, name="wcol")
                    nc.sync.dma_start(out=wcol[:], in_=w_col_ap(wi_, ot))
                    ps = qkv_psum.tile([128, TLOC], f32, name="ps_qk")
                    for ht in range(16):
                        nc.tensor.matmul(
                            ps[:], wcol[:, ht, :], x_int[:, ht, :],
                            start=(ht == 0), stop=(ht == 15),
                        )
                    nc.scalar.activation(
                        dst[:, ot, :], ps[:], AF.Identity,
                        scale=sxwb[:, scol : scol + 1], bias=bias_sb[:, ot : ot + 1],
                    )
                    nc.vector.tensor_reduce(
                        aparts[:, ot : ot + 1], dst[:, ot, :], AX.X, OP.max,
                        apply_absolute_value=True,
                    )

            # v in natural (t, o) orientation; load each Wv column-chunk once
            for oc in range(4):
                r0 = (oc * 512) // OSL  # 2 r-slices per 512-wide chunk
                wvms = []
                for ht in range(16):
                    wvm = wvm_pool.tile([128, 2, OSL], bf16, name="wvm")
                    nc.sync.dma_start(
                        out=wvm[:],
                        in_=wgs[2][r0 : r0 + 2].rearrange(
                            "r ht p o -> p ht r o"
                        )[:, ht],
                    )
                    wvms.append(wvm)
                for tt in range(4):
                    ps = qkv_psum.tile([128, TLOC], f32, name="ps_v")
                    for ht in range(16):
                        nc.tensor.matmul(
                            ps[:], x_int[:, ht, tt * 128 : (tt + 1) * 128],
                            wvms[ht][:].rearrange("p r o -> p (r o)"),
                            start=(ht == 0), stop=(ht == 15),
                        )
                    vtmp = qpool.tile([128, TLOC], f32, name="vtmp", tag="qs_f32")
                    nc.scalar.activation(vtmp[:], ps[:], AF.Copy, scale=sxwb[:, 2:3])
                    nc.vector.tensor_add(
                        v_deq[:, tt, oc * 512 : (oc + 1) * 512], vtmp[:],
                        bv_sb[:, oc * 512 : (oc + 1) * 512],
                    )
            for tt in range(4):
                nc.vector.tensor_reduce(
                    va_parts[:, tt : tt + 1], v_deq[:, tt, :], AX.X, OP.max,
                    apply_absolute_value=True,
                )

        for i, prt in enumerate((qa_parts, ka_parts, va_parts)):
            acol = pcols(1)
            nc.vector.tensor_reduce(acol, prt, AX.X, OP.max)
            ag = p_reduce_max(acol)
            nc.sync.dma_start(out=ar2_i[:, i : i + 1], in_=ag[0:1, :])
        nc.gpsimd.collective_compute(
            "AllReduce", OP.max, replica_groups=allg,
            ins=[ar2_i[:].opt()], outs=[ar2_o[:].opt()],
        )

        g3 = cols(3)[0:1, :]
        nc.sync.dma_start(out=g3, in_=ar2_o[:])
        s_q = cols(1)[0:1, :]
        nc.vector.tensor_scalar(s_q, g3[:, 0:1], Q_MUL / 127.0, 1e-8, op0=OP.mult, op1=OP.max)
        qf = cols(1)[0:1, :]
        nc.vector.reciprocal(qf, s_q)
        nc.vector.tensor_scalar_mul(qf, qf, Q_MUL)
        s_kv = cols(2)[0:1, :]
        nc.vector.tensor_scalar(s_kv, g3[:, 1:3], 1.0 / 127.0, 1e-8, op0=OP.mult, op1=OP.max)
        i_kv = cols(2)[0:1, :]
        nc.vector.reciprocal(i_kv, s_kv)
        s_qk = cols(1)[0:1, :]
        nc.vector.tensor_mul(s_qk, s_q, s_kv[:, 0:1])
        qf3 = cols(3)[0:1, :]
        nc.vector.tensor_copy(qf3[:, 0:1], qf)
        nc.vector.tensor_copy(qf3[:, 1:3], i_kv)
        qf3b = bcast(qf3)
        s_qk_b = bcast(s_qk)
        neg_inv_sqk = cols(1)[0:1, :]
        nc.vector.reciprocal(neg_inv_sqk, s_qk)
        nc.vector.tensor_scalar_mul(neg_inv_sqk, neg_inv_sqk, -1.0)
        nis_b = bcast(neg_inv_sqk)

        # quantize q, k, v; k/v -> bounce; AllGather within batch group
        kbs = [
            k_bs[i].rearrange("(ot p tl t) -> p ot tl t", p=128, t=128, ot=8)
            for i in range(2)
        ]
        for ot in range(16):
            m = qpool.tile([128, TLOC], f32, name="qm", tag="qs_f32")
            nc.scalar.activation(m[:], q_deq[:, ot, :], AF.Copy, scale=qf3b[:, 0:1])
            rnd(q_int[:, ot, :], m[:])
            if debug:
                nc.sync.dma_start(out=dbg["d_qdeq"].rearrange("a b c -> a b c")[:, ot, :], in_=q_deq[:, ot, :])
            m2 = qpool.tile([128, TLOC], f32, name="km", tag="qs_f32")
            nc.scalar.activation(m2[:], k_deq[:, ot, :], AF.Copy, scale=qf3b[:, 1:2])
            ki = qpool.tile([128, TLOC], bf16, name="ki", tag="qs_bf16")
            rnd(ki[:], m2[:])
            nc.sync.dma_start(
                out=kbs[ot // 8][:, ot % 8, :, :],
                in_=ki[:].rearrange("p (tl t) -> p tl t", t=128),
            )
            if ot == 7 or ot == 15:
                nc.gpsimd.collective_compute(
                    "AllGather", OP.bypass, replica_groups=grp,
                    ins=[k_bs[ot // 8][:].opt()], outs=[k_gs[ot // 8][:].opt()],
                )
            if debug:
                nc.sync.dma_start(out=dbg["d_kdeq"][:, ot, :], in_=k_deq[:, ot, :])
        vb = v_bounce.rearrange("(ot jp tt o) -> jp ot tt o", jp=128, o=128, tt=4)
        for tt in range(4):
            for oc in range(4):
                sl = slice(oc * 512, (oc + 1) * 512)
                m = qpool.tile([128, TLOC], f32, name="vm", tag="qs_f32")
                nc.scalar.activation(m[:], v_deq[:, tt, sl], AF.Copy, scale=qf3b[:, 2:3])
                vi = qpool.tile([128, TLOC], bf16, name="vi", tag="qs_bf16")
                rnd(vi[:], m[:])
                nc.sync.dma_start(
                    out=vb[:, oc * 4 : (oc + 1) * 4, tt, :],
                    in_=vi[:].rearrange("p (ot o) -> p ot o", o=128),
                )
            if debug:
                nc.sync.dma_start(out=dbg["d_vdeq"][:, tt, :], in_=v_deq[:, tt, :])
        v_deq_free()
        k_deq_free()
        q_deq_free()
        x_int_free()

        nc.gpsimd.collective_compute(
            "AllGather", OP.bypass, replica_groups=grp,
            ins=[v_bounce[:].opt()], outs=[v_g[:].opt()],
        )

        # gathered k: k_gs[h//8][r][ot'][p=d][tl][t'], j = r*512 + tl*128 + t'
        k_g_rs = [
            k_gs[i].rearrange("r (ot p tl t) -> ot p r tl t", p=128, t=128, ot=8)
            for i in range(2)
        ]

        def k_head_ap(h):  # (128 d, 4 r, 4 tl, 128 t)
            return k_g_rs[h // 8][h % 8]

        # gathered v: v_g[r][ot][jp][jtl][d'], j = r*512 + jtl*128 + jp
        v_g_r = v_g.rearrange("r (ot jp jtl o) -> jp r ot jtl o", jp=128, o=128, jtl=4)

        def v_head_ap(h):  # (128 jp, 4 r, 4 jtl, 128 d)
            return v_g_r[:, :, h, :, :]

        # ============ P4: attention pass 1 (stats) ============
        stats = sbuf.tile([128, 512], f32, name="stats")
        S_all = stats[:, 0:64]
        M_all = stats[:, 64:128]
        with tc.tile_pool(name="khead1", bufs=2) as khead_pool1, \
             tc.tile_pool(name="p4_psum", bufs=2, space="PSUM") as p4_psum, \
             tc.tile_pool(name="epool", bufs=2) as e_pool:
            for h in range(NH):
                khead = khead_pool1.tile([128, S], bf16, name="khead")
                for r in range(GROUP):
                    nc.sync.dma_start(
                        out=khead[:, r * 512 : (r + 1) * 512],
                        in_=k_head_ap(h)[:, r, :, :],
                    )
                for it in range(4):
                    ps = p4_psum.tile([128, S], f32, name="ps_a")
                    for jc in range(4):
                        nc.tensor.matmul(
                            ps[:, jc * 512 : (jc + 1) * 512],
                            q_int[:, h, it * 128 : (it + 1) * 128],
                            khead[:, jc * 512 : (jc + 1) * 512],
                            start=True, stop=True,
                        )
                    col = h * 4 + it
                    E = e_pool.tile([128, S], f32, name="E")
                    nc.scalar.activation(
                        E[:], ps[:], AF.Exp, scale=s_qk_b[:, 0:1],
                        accum_out=S_all[:, col : col + 1],
                    )
                    nc.vector.tensor_scalar(
                        E[:], E[:], 1.0, None, op0=OP.mult,
                        op1=OP.max, accum_out=M_all[:, col : col + 1],
                    )

        Sinv = stats[:, 128:192]
        nc.vector.reciprocal(Sinv, S_all)
        R = stats[:, 192:256]
        nc.vector.tensor_mul(R, M_all, Sinv)
        ra = pcols(1)
        nc.vector.tensor_reduce(ra, R, AX.X, OP.max)
        rag = p_reduce_max(ra)
        nc.sync.dma_start(out=ar3_i[:], in_=rag[0:1, :])
        nc.gpsimd.collective_compute(
            "AllReduce", OP.max, replica_groups=allg,
            ins=[ar3_i[:].opt()], outs=[ar3_o[:].opt()],
        )
        # A_i = -ln(S_i)/s_qk depends only on pass-1 stats: prepare DURING/before
        # AR3; the scalar -ln(s_attn) folds into the pass-2 exp bias instead.
        cl = stats[:, 320:384]
        nc.scalar.activation(cl, S_all, AF.Ln)
        cpr = stats[:, 384:448]
        nc.vector.tensor_scalar(cpr, cl, nis_b[:, 0:1], None, op0=OP.mult)
        chib = sbuf.tile([128, 64], bf16, name="chib")
        cmib = sbuf.tile([128, 64], bf16, name="cmib")
        clob = sbuf.tile([128, 64], bf16, name="clob")
        chif = stats[:, 448:512]
        res1 = stats[:, 128:192]  # reuse Sinv block (dead)
        res2 = stats[:, 192:256]  # reuse R block (dead)
        nc.vector.tensor_copy(chib[:], cpr)
        nc.vector.tensor_copy(chif, chib[:])
        nc.vector.tensor_sub(res1, cpr, chif)
        nc.vector.tensor_copy(cmib[:], res1)
        nc.vector.tensor_copy(chif, cmib[:])
        nc.vector.tensor_sub(res2, res1, chif)
        nc.vector.tensor_copy(clob[:], res2)
        for ci, t in ((0, chib), (1, cmib), (2, clob)):
            nc.sync.dma_start(
                out=rt_bf[ci].rearrange("h (it p) -> p h it", p=128),
                in_=t[:].rearrange("p (h it) -> p h it", it=4),
            )

        gA = cols(1)[0:1, :]
        nc.sync.dma_start(out=gA, in_=ar3_o[:])
        s_attn = cols(1)[0:1, :]
        nc.vector.tensor_scalar(s_attn, gA, 1.0 / 127.0, 1e-8, op0=OP.mult, op1=OP.max)
        # exp bias B = -ln(s_attn), broadcast per-partition
        lnsa = cols(1)[0:1, :]
        nc.scalar.activation(lnsa, s_attn, AF.Ln)
        nc.vector.tensor_scalar_mul(lnsa, lnsa, -1.0)
        eb_b = bcast(lnsa)
        s_av = cols(1)[0:1, :]
        nc.vector.tensor_mul(s_av, s_attn, s_kv[:, 1:2])
        s_av_b = bcast(s_av)

        if debug:
            nc.sync.dma_start(out=dbg["d_S"][:], in_=S_all)
            nc.sync.dma_start(out=dbg["d_M"][:], in_=M_all)

        # ============ P5: pass 2 + S@V ============
        out_T, out_T_free = tc.tile([128, 16, TLOC], f32, name="out_T")
        oa_parts = pcols(16)
        with tc.tile_pool(name="khead2", bufs=2) as khead_pool2, \
             tc.tile_pool(name="vhead", bufs=3) as vhead_pool, \
             tc.tile_pool(name="p5_psum", bufs=3, space="PSUM") as p5_psum, \
             tc.tile_pool(name="sv_psum", bufs=2, space="PSUM") as sv_psum, \
             tc.tile_pool(name="pint", bufs=3) as pint_pool, \
             tc.tile_pool(name="ps_scr", bufs=3) as ps_scr, \
             tc.tile_pool(name="cpool", bufs=2) as cpool:
            pints = {}

            def sv_head(h):
                p_int, vhead = pints.pop(h)
                ps3 = sv_psum.tile([128, TLOC], f32, name="ps3")
                for jt in range(16):
                    nc.tensor.matmul(
                        ps3[:], vhead[:, jt, :], p_int[:, jt, :],
                        start=(jt == 0), stop=(jt == 15),
                    )
                nc.scalar.activation(out_T[:, h, :], ps3[:], AF.Copy, scale=s_av_b[:, 0:1])
                nc.vector.tensor_reduce(
                    oa_parts[:, h : h + 1], out_T[:, h, :], AX.X, OP.max,
                    apply_absolute_value=True,
                )
                if debug and h == 0:
                    nc.sync.dma_start(out=dbg["d_pint"][:], in_=p_int[:])

            for h in range(NH):
                khead = khead_pool2.tile([128, S], bf16, name="khead")
                for r in range(GROUP):
                    nc.sync.dma_start(
                        out=khead[:, r * 512 : (r + 1) * 512],
                        in_=k_head_ap(h)[:, r, :, :],
                    )
                cmv = cpool.tile([3, 512], bf16, name="cmv")
                nc.sync.dma_start(out=cmv[:], in_=rt_bf[:, h, :])
                vhead = vhead_pool.tile([128, 16, 128], bf16, name="vhead")
                for r in range(GROUP):
                    nc.sync.dma_start(
                        out=vhead[:, r * 4 : (r + 1) * 4, :],
                        in_=v_head_ap(h)[:, r, :, :],
                    )
                p_int = pint_pool.tile([128, 16, TLOC], bf16, name="p_int")
                for jp in range(8):
                    ps2 = p5_psum.tile([128, 2 * TLOC], f32, name="ps2")
                    for half in range(2):
                        jt = jp * 2 + half
                        sl = slice(half * TLOC, (half + 1) * TLOC)
                        nc.tensor.matmul(
                            ps2[:, sl], khead[:, jt * 128 : (jt + 1) * 128],
                            q_int[:, h, :], start=True, stop=False,
                        )
                    for half in range(2):
                        sl = slice(half * TLOC, (half + 1) * TLOC)
                        nc.tensor.matmul(
                            ps2[:, sl], ones3[:], cmv[:],
                            start=False, stop=True, skip_group_check=True,
                        )
                    PS = ps_scr.tile([128, 2 * TLOC], f32, name="PS")
                    nc.scalar.activation(
                        PS[:], ps2[:], AF.Exp,
                        scale=s_qk_b[:, 0:1], bias=eb_b[:, 0:1],
                    )
                    rnd(
                        p_int[:, jp * 2 : jp * 2 + 2, :].rearrange("p a b -> p (a b)"),
                        PS[:],
                    )
                pints[h] = (p_int, vhead)
                if h > 0:
                    sv_head(h - 1)
            sv_head(NH - 1)

        # ============ P6: out amax -> AR4 -> quantize ============
        oc_ = pcols(1)
        nc.vector.tensor_reduce(oc_, oa_parts, AX.X, OP.max)
        ocg = p_reduce_max(oc_)
        nc.sync.dma_start(out=ar4_i[:], in_=ocg[0:1, :])
        nc.gpsimd.collective_compute(
            "AllReduce", OP.max, replica_groups=allg,
            ins=[ar4_i[:].opt()], outs=[ar4_o[:].opt()],
        )
        gO = cols(1)[0:1, :]
        nc.sync.dma_start(out=gO, in_=ar4_o[:])
        s_out = cols(1)[0:1, :]
        nc.vector.tensor_scalar(s_out, gO, 1.0 / 127.0, 1e-8, op0=OP.mult, op1=OP.max)
        i_out = cols(1)[0:1, :]
        nc.vector.reciprocal(i_out, s_out)
        io_b = bcast(i_out)
        s_op = cols(1)[0:1, :]
        nc.vector.tensor_mul(s_op, s_out, s5[:, 4:5])
        s_op_b = bcast(s_op)

        out_int, out_int_free = tc.tile([128, 16, TLOC], bf16, name="out_int")
        for ot in range(16):
            m = qpool.tile([128, TLOC], f32, name="om", tag="qs_f32")
            nc.scalar.activation(m[:], out_T[:, ot, :], AF.Copy, scale=io_b[:, 0:1])
            rnd(out_int[:, ot, :], m[:])
            if debug:
                nc.sync.dma_start(out=dbg["d_outT"][:, ot, :], in_=out_T[:, ot, :])

        # ============ P7: output projection ============
        out_r = out_ext.rearrange("(ot p) t -> p ot t", p=128)
        with tc.tile_pool(name="p7_psum", bufs=4, space="PSUM") as p7_psum, \
             tc.tile_pool(name="wcol7", bufs=3) as wcol_pool7, \
             tc.tile_pool(name="fin", bufs=3) as fin_pool:
            for ot in range(16):
                wcol = wcol_pool7.tile([128, 16, 128], bf16, name="wcol")
                nc.sync.dma_start(out=wcol[:], in_=w_col_ap(3, ot))
                ps = p7_psum.tile([128, TLOC], f32, name="ps_p")
                for ht in range(16):
                    nc.tensor.matmul(
                        ps[:], wcol[:, ht, :], out_int[:, ht, :],
                        start=(ht == 0), stop=(ht == 15),
                    )
                fin = fin_pool.tile([128, TLOC], f32, name="fin")
                nc.scalar.activation(
                    fin[:], ps[:], AF.Identity,
                    scale=s_op_b[:, 0:1], bias=bp_sb[:, ot : ot + 1],
                )
                nc.sync.dma_start(out=out_r[:, ot, :], in_=fin[:])
        out_int_free()
        out_T_free()
        q_int_free()

        if debug:
            sct = cols(16)[0:1, :]
            nc.vector.tensor_copy(sct[:, 0:5], s5)
            nc.vector.tensor_copy(sct[:, 5:6], s_q)
            nc.vector.tensor_copy(sct[:, 6:8], s_kv)
            nc.vector.tensor_copy(sct[:, 8:9], s_attn)
            nc.vector.tensor_copy(sct[:, 9:10], s_out)
            nc.sync.dma_start(out=dbg["d_scales"][:], in_=sct)

    nc.compile()
    return nc


def _get_compiled(debug=False):
    if debug not in _COMPILED:
        _COMPILED[debug] = _build(debug)
    return _COMPILED[debug]


def make_in_maps(hidden_states, Wq, bq, Wk, bk, Wv, bv, Wp, bp):
    hs = np.asarray(hidden_states, dtype=np.float32)
    wT = [
        np.ascontiguousarray(np.asarray(W, np.float32).T)
        for W in (Wq, Wk, Wv, Wp)
    ]
    bq_t = np.ascontiguousarray(np.asarray(bq, np.float32).reshape(16, 128).T)
    bk_t = np.ascontiguousarray(np.asarray(bk, np.float32).reshape(16, 128).T)
    bp_t = np.ascontiguousarray(np.asarray(bp, np.float32).reshape(16, 128).T)
    bv_b = np.ascontiguousarray(
        np.broadcast_to(np.asarray(bv, np.float32)[None, :], (128, H))
    )
    in_maps = []
    for c in range(NC):
        b = c // GROUP
        t0 = (c % GROUP) * TLOC
        x_Tc = np.ascontiguousarray(hs[b, t0 : t0 + TLOC, :].T)
        wslc = np.ascontiguousarray(
            np.stack([wT[w][:, c * OSL : (c + 1) * OSL] for w in range(4)], axis=0)
        )
        in_maps.append(
            {"x_T": x_Tc, "wsl": wslc, "bq_t": bq_t, "bk_t": bk_t,
             "bp_t": bp_t, "bv_b": bv_b}
        )
    return in_maps


def kernel(hidden_states, Wq, bq, Wk, bk, Wv, bv, Wp, bp):
    from concourse.bass_utils import run_bass_kernel_spmd

    debug = bool(int(os.environ.get("KERNEL_DEBUG", "0")))
    trace = bool(int(os.environ.get("KERNEL_TRACE", "0")))
    nc = _get_compiled(debug=debug)
    in_maps = make_in_maps(hidden_states, Wq, bq, Wk, bk, Wv, bv, Wp, bp)
    res = run_bass_kernel_spmd(nc, in_maps, core_ids=list(range(NC)), trace=trace)
    kernel.last_exec_time_ns = res.exec_time_ns
    kernel.last_results = res.results
    kernel.last_res = res

    out = np.empty((B, S, H), dtype=np.float32)
    for c in range(NC):
        b = c // GROUP
        t0 = (c % GROUP) * TLOC
        out[b, t0 : t0 + TLOC, :] = res.results[c]["out"].T
    return out


kernel.last_exec_time_ns = None
kernel.last_results = None



# revision 15
# speedup vs baseline: 1.1205x; 1.1205x over previous
"""Distributed TRN2 Bass kernel for a dynamic-int8-quantized transformer encoder.

v2: column-parallel sharding, 2 heads x both batches per core.
  - core c: heads {2c, 2c+1} (256 qkv out dims), io token slice = flat block c
    (batch c//4, tokens (c%4)*512..+512).
  - x quantized per-core on its 512-token slice, AllGathered all-8 (RDH) as
    integer-valued bf16; Wq/Wk/Wv column slices stay LOCAL (no weight
    AllGather); Wp quantized sharded + AllGathered (overlapped mid-kernel).
  - projections column-parallel: q/k/v for ALL 4096 tokens x this core's
    2 heads; attention fully local per (batch, head) - 4 pairs per core.
  - after attention: AllToAll (all-8) redistributes quantized out to
    token-parallel; final projection token-parallel vs gathered Wp.
  - int8 fake-quant matmuls computed exactly as integer-valued bf16 matmuls
    (fp32 PSUM), dequant scales on PSUM eviction; global per-tensor amaxes via
    tiny AllReduce-max collectives.
  - softmax without max-subtraction; pass2 recomputes QK^T transposed with an
    extra k=3 ones-matmul adding c'_i = -ln(S_i)/s_qk (bf16 hi+mid+lo split),
    so round(exp(s_qk*(a_int + c'_i) - ln s_attn)) yields int8 probs directly.
  - round-to-nearest-even: x + 1.5*2^23 - 1.5*2^23 in fp32 (matches jnp.round).
"""

import math
import os
import sys
from contextlib import ExitStack

import numpy as np

sys.path.insert(0, "/opt/trn_rl_repo")

B = 2
S = 2048
H = 2048
NH = 16
D = 128
NC = 8
GROUP = 4          # io-slice cores per batch
HPC = 2            # heads per core
OS = 256           # per-core qkv out-dim slice (2 heads)
NT = 4096          # total tokens (B*S)
TB = 2048          # tokens per batch
TLOC = 512         # io token slice per core
OSL = 256          # per-core Wp out-dim slice
Q_MUL = 1.0 / math.sqrt(D)
RMAGIC = 12582912.0  # 1.5 * 2**23

_COMPILED = {}


def _build(debug=False):
    import concourse.mybir as mybir
    import concourse.tile as tile
    from concourse import bacc
    from concourse import bass_isa

    f32 = mybir.dt.float32
    bf16 = mybir.dt.bfloat16
    AF = mybir.ActivationFunctionType
    OP = mybir.AluOpType
    AX = mybir.AxisListType

    nc = bacc.Bacc(None, target_bir_lowering=False, debug=False, num_devices=NC)

    x_T = nc.declare_dram_parameter("x_T", [H, TLOC], f32, isOutput=False)
    wqkv = nc.declare_dram_parameter("wqkv", [3, H, OS], f32, isOutput=False)
    wp_sl = nc.declare_dram_parameter("wp_sl", [H, OSL], f32, isOutput=False)
    bqk = nc.declare_dram_parameter("bqk", [2, 128, HPC], f32, isOutput=False)
    bv_b = nc.declare_dram_parameter("bv_b", [128, OS], f32, isOutput=False)
    bp_t = nc.declare_dram_parameter("bp_t", [128, 16], f32, isOutput=False)
    out_ext = nc.declare_dram_parameter("out", [H, TLOC], f32, isOutput=True)
    dbg = {}
    if debug:
        for name, shape in [
            ("d_scales", [1, 16]),
            ("d_S", [128, 64]), ("d_M", [128, 64]),
            ("d_qdeq", [128, HPC, NT]), ("d_kdeq", [128, HPC, NT]),
            ("d_vdeq", [32, 128, OS]), ("d_outT", [128, HPC, NT]),
        ]:
            dbg[name] = nc.declare_dram_parameter(name, shape, f32, isOutput=True)

    allg = [list(range(NC))]

    with tile.TileContext(nc) as tc, ExitStack() as top:
        dram = top.enter_context(tc.tile_pool(name="dram", bufs=1, space="DRAM"))
        # x AllGather bounces (2 chunks of 8 ht each), all-8 gather
        xq_b = [dram.tile([8, 128, TLOC], bf16, name=f"xqb{i}") for i in range(2)]
        xg = [
            dram.tile([NC, 8, 128, TLOC], bf16, addr_space="Shared", name=f"xg{i}")
            for i in range(2)
        ]
        # wv bounce (streamed back during v projection)
        wvb = dram.tile([16, 128, OS], bf16, name="wvb")
        # v_deq spill (f32, streamed back for quantization after AR2)
        vdq = dram.tile([32, 128, OS], f32, name="vdq")
        # wp bounce + gather (stationary-tiled 4KB read rows)
        wpb = dram.tile([2, 128, 16, 128], bf16, name="wpb")
        wpg = dram.tile([NC, 2, 128, 16, 128], bf16, addr_space="Shared", name="wpg")
        # A2A for out_int redistribution (all-8)
        a2a_i = dram.tile([NC, HPC, 128, TLOC], bf16, name="a2ai")
        a2a_o = dram.tile([NC, HPC, 128, TLOC], bf16, name="a2ao")
        # c' rows for pass2 ones-matmul: [3][bh-pair][i]
        rt_bf = dram.tile([3, 4, TB], bf16, name="rtbf")
        # AllReduce bounces
        ar1a_i = dram.tile([1, 3], f32)
        ar1a_o = dram.tile([1, 3], f32, addr_space="Shared")
        ar1b_i = dram.tile([1, 2], f32)
        ar1b_o = dram.tile([1, 2], f32, addr_space="Shared")
        ar2_i = dram.tile([1, 3], f32)
        ar2_o = dram.tile([1, 3], f32, addr_space="Shared")
        ar3_i = dram.tile([1, 1], f32)
        ar3_o = dram.tile([1, 1], f32, addr_space="Shared")
        ar4_i = dram.tile([1, 1], f32)
        ar4_o = dram.tile([1, 1], f32, addr_space="Shared")

        const = top.enter_context(tc.tile_pool(name="const", bufs=1))
        sc = top.enter_context(tc.tile_pool(name="scal", bufs=1))
        sbuf = top.enter_context(tc.tile_pool(name="sbuf_main", bufs=1))
        qpool = top.enter_context(tc.tile_pool(name="qscratch", bufs=3))

        ones3 = const.tile([3, 128], bf16)
        nc.vector.memset(ones3[:], 1.0)

        scal = sc.tile([128, 96], f32, name="scal")
        _col = [0]

        def cols(n):
            c0 = _col[0]
            _col[0] += n
            assert _col[0] <= 96
            return scal[:, c0 : c0 + n]

        parts = sc.tile([128, 192], f32, name="parts")
        _pcol = [0]

        def pcols(n):
            c0 = _pcol[0]
            _pcol[0] += n
            assert _pcol[0] <= 192
            return parts[:, c0 : c0 + n]

        def p_reduce_max(part_col):
            red = cols(1)
            nc.gpsimd.partition_all_reduce(
                red, part_col, channels=128, reduce_op=bass_isa.ReduceOp.max
            )
            return red

        def bcast(src1n):
            b = cols(src1n.shape[-1])
            nc.gpsimd.partition_broadcast(b, src1n)
            return b

        def rnd(out_ap, in_ap):
            nc.vector.tensor_scalar(
                out_ap, in_ap, RMAGIC, RMAGIC, op0=OP.add, op1=OP.subtract
            )

        # persistent tiles (stack order: freed in reverse)
        q_int, q_int_free = tc.tile([128, HPC, NT], bf16, name="q_int")
        k_int, k_int_free = tc.tile([128, HPC, NT], bf16, name="k_int")
        wq_int, wq_int_free = tc.tile([128, 16, OS], bf16, name="wq_int")
        wk_int, wk_int_free = tc.tile([128, 16, OS], bf16, name="wk_int")

        # ============ S0: load x slice; amaxes of x and W slices; AR1 ============
        wqkv_r = wqkv.rearrange("w (ht p) o -> w p ht o", p=128)
        wp_r = wp_sl.rearrange("(ht p) o -> p ht o", p=128)

        with tc.tile_pool(name="xf", bufs=1) as xf_pool, \
             tc.tile_pool(name="wf", bufs=4) as wf_pool:
            x_f32 = xf_pool.tile([128, 16, TLOC], f32, name="x_f32")
            nc.sync.dma_start(out=x_f32[:], in_=x_T.rearrange("(ht p) t -> p ht t", p=128))
            xparts = pcols(16)
            for ht in range(16):
                nc.vector.tensor_reduce(
                    xparts[:, ht : ht + 1], x_f32[:, ht, :], AX.X, OP.max,
                    apply_absolute_value=True,
                )
            xa = pcols(1)
            nc.vector.tensor_reduce(xa, xparts, AX.X, OP.max)
            xag = p_reduce_max(xa)
            nc.sync.dma_start(out=ar1a_i[:, 0:1], in_=xag[0:1, :])

            def w_amax(src_ht_ap, dst_ap):
                wap = pcols(16)
                for ht in range(16):
                    wt = wf_pool.tile([128, OS], f32, name="wt", tag="wtf")
                    nc.sync.dma_start(out=wt[:], in_=src_ht_ap(ht))
                    nc.vector.tensor_reduce(
                        wap[:, ht : ht + 1], wt[:], AX.X, OP.max,
                        apply_absolute_value=True,
                    )
                wa = pcols(1)
                nc.vector.tensor_reduce(wa, wap, AX.X, OP.max)
                wag = p_reduce_max(wa)
                nc.sync.dma_start(out=dst_ap, in_=wag[0:1, :])

            # AR1a: x, wq, wk
            w_amax(lambda ht: wqkv_r[0, :, ht, :], ar1a_i[:, 1:2])
            w_amax(lambda ht: wqkv_r[1, :, ht, :], ar1a_i[:, 2:3])
            nc.gpsimd.collective_compute(
                "AllReduce", OP.max, replica_groups=allg,
                ins=[ar1a_i[:].opt()], outs=[ar1a_o[:].opt()],
            )
            # AR1b: wv, wp
            w_amax(lambda ht: wqkv_r[2, :, ht, :], ar1b_i[:, 0:1])
            w_amax(lambda ht: wp_r[:, ht, :], ar1b_i[:, 1:2])
            nc.gpsimd.collective_compute(
                "AllReduce", OP.max, replica_groups=allg,
                ins=[ar1b_i[:].opt()], outs=[ar1b_o[:].opt()],
            )

            # scales: slots [x, wq, wk, wv, wp]
            g5 = cols(5)[0:1, :]
            s5 = cols(5)[0:1, :]
            i5 = cols(5)[0:1, :]
            i5b = cols(5)
            sxw = cols(3)[0:1, :]   # s_x*s_wq, s_x*s_wk, s_x*s_wv
            sxwb = cols(3)

            nc.sync.dma_start(out=g5[:, 0:3], in_=ar1a_o[:])
            nc.sync.dma_start(out=g5[:, 3:5], in_=ar1b_o[:])
            nc.vector.tensor_scalar(
                s5, g5, 1.0 / 127.0, 1e-8, op0=OP.mult, op1=OP.max
            )
            nc.vector.reciprocal(i5, s5)
            nc.gpsimd.partition_broadcast(i5b, i5)
            nc.vector.tensor_mul(sxw[:, 0:1], s5[:, 0:1], s5[:, 1:2])
            nc.vector.tensor_mul(sxw[:, 1:2], s5[:, 0:1], s5[:, 2:3])
            nc.vector.tensor_mul(sxw[:, 2:3], s5[:, 0:1], s5[:, 3:4])
            nc.gpsimd.partition_broadcast(sxwb, sxw)

            # ============ S1: quantize x slice -> bounce -> AGx (2 chunks) =======
            for ht in range(16):
                xm = qpool.tile([128, TLOC], f32, name="xm", tag="qs_f32")
                nc.scalar.activation(xm[:], x_f32[:, ht, :], AF.Copy, scale=i5b[:, 0:1])
                xi = qpool.tile([128, TLOC], bf16, name="xi", tag="qs_bf16")
                rnd(xi[:], xm[:])
                nc.sync.dma_start(out=xq_b[ht // 8][ht % 8], in_=xi[:])
                if ht == 7 or ht == 15:
                    nc.gpsimd.collective_compute(
                        "AllGather", OP.bypass, replica_groups=allg,
                        ins=[xq_b[ht // 8][:].opt()], outs=[xg[ht // 8][:].opt()],
                    )

            # ============ S2: quantize weights ============
            for w, dst in ((0, wq_int), (1, wk_int)):
                for ht in range(16):
                    wt = wf_pool.tile([128, OS], f32, name="wt2", tag="wtf")
                    nc.sync.dma_start(out=wt[:], in_=wqkv_r[w, :, ht, :])
                    wm = qpool.tile([128, OS], f32, name="wm", tag="qs_f32")
                    nc.scalar.activation(wm[:], wt[:], AF.Copy, scale=i5b[:, 1 + w : 2 + w])
                    rnd(dst[:, ht, :], wm[:])
            # wv -> DRAM bounce (streamed back at v projection)
            for ht in range(16):
                wt = wf_pool.tile([128, OS], f32, name="wt3", tag="wtf")
                nc.sync.dma_start(out=wt[:], in_=wqkv_r[2, :, ht, :])
                wm = qpool.tile([128, OS], f32, name="wm3", tag="qs_f32")
                nc.scalar.activation(wm[:], wt[:], AF.Copy, scale=i5b[:, 3:4])
                wi = qpool.tile([128, OS], bf16, name="wi3", tag="qs_bf16")
                rnd(wi[:], wm[:])
                nc.sync.dma_start(out=wvb[ht], in_=wi[:])
            # wp slice -> DRAM bounce (AllGather queued after AR2)
            wpb_r = wpb.rearrange("half p ht o -> p half ht o")
            for ht in range(16):
                wt = wf_pool.tile([128, OS], f32, name="wt4", tag="wtf")
                nc.sync.dma_start(out=wt[:], in_=wp_r[:, ht, :])
                wm = qpool.tile([128, OS], f32, name="wm4", tag="qs_f32")
                nc.scalar.activation(wm[:], wt[:], AF.Copy, scale=i5b[:, 4:5])
                wi = qpool.tile([128, OS], bf16, name="wi4", tag="qs_bf16")
                rnd(wi[:], wm[:])
                nc.sync.dma_start(
                    out=wpb_r[:, :, ht, :],
                    in_=wi[:].rearrange("p (half o) -> p half o", o=128),
                )

        # ============ S4: projections (v, q, k) column-parallel ============
        q_deq, q_deq_free = tc.tile([128, HPC, NT], f32, name="q_deq")
        k_deq, k_deq_free = tc.tile([128, HPC, NT], f32, name="k_deq")
        bq_sb = const.tile([128, HPC], f32)
        nc.sync.dma_start(out=bq_sb[:], in_=bqk[0])
        bk_sb = const.tile([128, HPC], f32)
        nc.sync.dma_start(out=bk_sb[:], in_=bqk[1])
        bv_sb = const.tile([128, OS], f32)
        nc.sync.dma_start(out=bv_sb[:], in_=bv_b[:, :])
        bp_sb = const.tile([128, 16], f32)
        nc.sync.dma_start(out=bp_sb[:], in_=bp_t[:, :])

        qa_parts = pcols(16)
        ka_parts = pcols(16)
        va_parts = pcols(32)

        with tc.tile_pool(name="xt_lo", bufs=2) as xlo_pool, \
             tc.tile_pool(name="xt_hi", bufs=2) as xhi_pool, \
             tc.tile_pool(name="wvm", bufs=18) as wvm_pool, \
             tc.tile_pool(name="v_psum", bufs=2, space="PSUM") as v_psum, \
             tc.tile_pool(name="qk_psum", bufs=4, space="PSUM") as qk_psum:
            for tt in range(NC):
                xlo = xlo_pool.tile([128, 8, TLOC], bf16, name="xlo")
                xhi = xhi_pool.tile([128, 8, TLOC], bf16, name="xhi")
                nc.sync.dma_start(
                    out=xlo[:], in_=xg[0][tt].rearrange("h p t -> p h t")
                )
                nc.sync.dma_start(
                    out=xhi[:], in_=xg[1][tt].rearrange("h p t -> p h t")
                )
                xt = (xlo, xhi)

                def xtile(ht):
                    return xt[ht // 8][:, ht % 8, :]

                # v first: out [tok, o] per 128-token block
                wvms = []
                for ht in range(16):
                    wvm = wvm_pool.tile([128, OS], bf16, name="wvm")
                    nc.sync.dma_start(out=wvm[:], in_=wvb[ht])
                    wvms.append(wvm)
                for tc4 in range(4):
                    ps = v_psum.tile([128, OS], f32, name="ps_v")
                    for ht in range(16):
                        nc.tensor.matmul(
                            ps[:], xtile(ht)[:, tc4 * 128 : (tc4 + 1) * 128],
                            wvms[ht][:],
                            start=(ht == 0), stop=(ht == 15),
                        )
                    gt = tt * 4 + tc4
                    vtmp = qpool.tile([128, OS], f32, name="vtmp", tag="qs_f32")
                    nc.scalar.activation(vtmp[:], ps[:], AF.Copy, scale=sxwb[:, 2:3])
                    vdq_t = qpool.tile([128, OS], f32, name="vdqt", tag="qs_f32b")
                    nc.vector.tensor_add(vdq_t[:], vtmp[:], bv_sb[:])
                    nc.vector.tensor_reduce(
                        va_parts[:, gt : gt + 1], vdq_t[:], AX.X, OP.max,
                        apply_absolute_value=True,
                    )
                    nc.sync.dma_start(out=vdq[gt], in_=vdq_t[:])
                # q, k
                for w, wint, dst, bias_sb, scol, aparts in (
                    (0, wq_int, q_deq, bq_sb, 0, qa_parts),
                    (1, wk_int, k_deq, bk_sb, 1, ka_parts),
                ):
                    for ot in range(HPC):
                        ps = qk_psum.tile([128, TLOC], f32, name="ps_qk")
                        for ht in range(16):
                            nc.tensor.matmul(
                                ps[:], wint[:, ht, ot * 128 : (ot + 1) * 128],
                                xtile(ht),
                                start=(ht == 0), stop=(ht == 15),
                            )
                        nc.scalar.activation(
                            dst[:, ot, tt * TLOC : (tt + 1) * TLOC], ps[:],
                            AF.Identity, scale=sxwb[:, scol : scol + 1],
                            bias=bias_sb[:, ot : ot + 1],
                        )
                        nc.vector.tensor_reduce(
                            qa_parts[:, tt * 2 + ot : tt * 2 + ot + 1]
                            if w == 0
                            else ka_parts[:, tt * 2 + ot : tt * 2 + ot + 1],
                            dst[:, ot, tt * TLOC : (tt + 1) * TLOC], AX.X, OP.max,
                            apply_absolute_value=True,
                        )

        for i, prt in enumerate((qa_parts, ka_parts, va_parts)):
            acol = pcols(1)
            nc.vector.tensor_reduce(acol, prt, AX.X, OP.max)
            ag = p_reduce_max(acol)
            nc.sync.dma_start(out=ar2_i[:, i : i + 1], in_=ag[0:1, :])
        nc.gpsimd.collective_compute(
            "AllReduce", OP.max, replica_groups=allg,
            ins=[ar2_i[:].opt()], outs=[ar2_o[:].opt()],
        )

        # scales from AR2: [q, k, v]
        g3 = cols(3)[0:1, :]
        nc.sync.dma_start(out=g3, in_=ar2_o[:])
        s_q = cols(1)[0:1, :]
        nc.vector.tensor_scalar(s_q, g3[:, 0:1], Q_MUL / 127.0, 1e-8, op0=OP.mult, op1=OP.max)
        qf = cols(1)[0:1, :]
        nc.vector.reciprocal(qf, s_q)
        nc.vector.tensor_scalar_mul(qf, qf, Q_MUL)
        s_kv = cols(2)[0:1, :]
        nc.vector.tensor_scalar(s_kv, g3[:, 1:3], 1.0 / 127.0, 1e-8, op0=OP.mult, op1=OP.max)
        i_kv = cols(2)[0:1, :]
        nc.vector.reciprocal(i_kv, s_kv)
        s_qk = cols(1)[0:1, :]
        nc.vector.tensor_mul(s_qk, s_q, s_kv[:, 0:1])
        qf3 = cols(3)[0:1, :]
        nc.vector.tensor_copy(qf3[:, 0:1], qf)
        nc.vector.tensor_copy(qf3[:, 1:3], i_kv)
        qf3b = bcast(qf3)
        s_qk_b = bcast(s_qk)
        neg_inv_sqk = cols(1)[0:1, :]
        nc.vector.reciprocal(neg_inv_sqk, s_qk)
        nc.vector.tensor_scalar_mul(neg_inv_sqk, neg_inv_sqk, -1.0)
        nis_b = bcast(neg_inv_sqk)

        # wp AllGather: queued after the AR2 scale broadcasts so it cannot
        # stall them on the gpsimd queue; drains during quantize/pass1.
        nc.gpsimd.collective_compute(
            "AllGather", OP.bypass, replica_groups=allg,
            ins=[wpb[:].opt()], outs=[wpg[:].opt()],
        )

        # ============ S5: quantize q, k, v ============
        with tc.tile_pool(name="q5scratch", bufs=2) as q5pool:
            for ot in range(HPC):
                for bh in range(2):
                    tsl = slice(bh * TB, (bh + 1) * TB)
                    m = q5pool.tile([128, TB], f32, name="qm", tag="qs2_f32")
                    nc.scalar.activation(m[:], q_deq[:, ot, tsl], AF.Copy, scale=qf3b[:, 0:1])
                    rnd(q_int[:, ot, tsl], m[:])
                    m2 = q5pool.tile([128, TB], f32, name="km", tag="qs2_f32")
                    nc.scalar.activation(m2[:], k_deq[:, ot, tsl], AF.Copy, scale=qf3b[:, 1:2])
                    rnd(k_int[:, ot, tsl], m2[:])
                if debug:
                    nc.sync.dma_start(out=dbg["d_qdeq"][:, ot, :], in_=q_deq[:, ot, :])
                    nc.sync.dma_start(out=dbg["d_kdeq"][:, ot, :], in_=k_deq[:, ot, :])
        k_deq_free()
        q_deq_free()
        wk_int_free()
        wq_int_free()
        v_int, v_int_free = tc.tile([128, 32, OS], bf16, name="v_int")
        with tc.tile_pool(name="v5scratch", bufs=2) as v5pool:
            for g8 in range(4):
                vsl8 = slice(g8 * 8, (g8 + 1) * 8)
                vback = v5pool.tile([128, TB], f32, name="vback", tag="vs2_f32")
                nc.sync.dma_start(
                    out=vback[:].rearrange("p (g o) -> p g o", o=OS),
                    in_=vdq[vsl8].rearrange("g p o -> p g o"),
                )
                m = v5pool.tile([128, TB], f32, name="vm", tag="vs2_f32")
                nc.scalar.activation(m[:], vback[:], AF.Copy, scale=qf3b[:, 2:3])
                rnd(
                    v_int[:, vsl8, :].rearrange("p a b -> p (a b)"),
                    m[:],
                )
        if debug:
            nc.sync.dma_start(out=dbg["d_vdeq"][:], in_=vdq[:])

        # ============ S6: attention pass 1 (stats) ============
        # pair bp_ = (b, h): b = bp_//2, h = bp_%2
        stats = sbuf.tile([128, 512], f32, name="stats")
        S_all = stats[:, 0:64]
        M_all = stats[:, 64:128]
        with tc.tile_pool(name="p1_psum", bufs=2, space="PSUM") as p1_psum, \
             tc.tile_pool(name="epool", bufs=2) as e_pool:
            for bp_ in range(4):
                b_, h_ = bp_ // 2, bp_ % 2
                tb0 = b_ * TB
                for it in range(16):
                    ps = p1_psum.tile([128, TB], f32, name="ps_a")
                    for jc in range(4):
                        nc.tensor.matmul(
                            ps[:, jc * 512 : (jc + 1) * 512],
                            q_int[:, h_, tb0 + it * 128 : tb0 + (it + 1) * 128],
                            k_int[:, h_, tb0 + jc * 512 : tb0 + (jc + 1) * 512],
                            start=True, stop=True,
                        )
                    col = bp_ * 16 + it
                    E = e_pool.tile([128, TB], f32, name="E")
                    nc.scalar.activation(
                        E[:], ps[:], AF.Exp, scale=s_qk_b[:, 0:1],
                        accum_out=S_all[:, col : col + 1],
                    )
                    nc.vector.tensor_reduce(
                        M_all[:, col : col + 1], E[:], AX.X, OP.max,
                    )

        # ============ S7: AR3 + c' rows ============
        Sinv = stats[:, 128:192]
        nc.vector.reciprocal(Sinv, S_all)
        R = stats[:, 192:256]
        nc.vector.tensor_mul(R, M_all, Sinv)
        ra = pcols(1)
        nc.vector.tensor_reduce(ra, R, AX.X, OP.max)
        rag = p_reduce_max(ra)
        nc.sync.dma_start(out=ar3_i[:], in_=rag[0:1, :])
        nc.gpsimd.collective_compute(
            "AllReduce", OP.max, replica_groups=allg,
            ins=[ar3_i[:].opt()], outs=[ar3_o[:].opt()],
        )
        # c'_i = -ln(S_i)/s_qk as bf16 hi+mid+lo, prepared during AR3
        cl = stats[:, 320:384]
        nc.scalar.activation(cl, S_all, AF.Ln)
        cpr = stats[:, 384:448]
        nc.vector.tensor_scalar(cpr, cl, nis_b[:, 0:1], None, op0=OP.mult)
        chib = sbuf.tile([128, 64], bf16, name="chib")
        cmib = sbuf.tile([128, 64], bf16, name="cmib")
        clob = sbuf.tile([128, 64], bf16, name="clob")
        chif = stats[:, 448:512]
        res1 = stats[:, 128:192]
        res2 = stats[:, 192:256]
        nc.vector.tensor_copy(chib[:], cpr)
        nc.vector.tensor_copy(chif, chib[:])
        nc.vector.tensor_sub(res1, cpr, chif)
        nc.vector.tensor_copy(cmib[:], res1)
        nc.vector.tensor_copy(chif, cmib[:])
        nc.vector.tensor_sub(res2, res1, chif)
        nc.vector.tensor_copy(clob[:], res2)
        for ci, t in ((0, chib), (1, cmib), (2, clob)):
            nc.sync.dma_start(
                out=rt_bf[ci].rearrange("h (it p) -> p h it", p=128),
                in_=t[:].rearrange("p (h it) -> p h it", it=16),
            )

        gA = cols(1)[0:1, :]
        nc.sync.dma_start(out=gA, in_=ar3_o[:])
        s_attn = cols(1)[0:1, :]
        nc.vector.tensor_scalar(s_attn, gA, 1.0 / 127.0, 1e-8, op0=OP.mult, op1=OP.max)
        lnsa = cols(1)[0:1, :]
        nc.scalar.activation(lnsa, s_attn, AF.Ln)
        nc.vector.tensor_scalar_mul(lnsa, lnsa, -1.0)
        eb_b = bcast(lnsa)
        s_av = cols(1)[0:1, :]
        nc.vector.tensor_mul(s_av, s_attn, s_kv[:, 1:2])
        s_av_b = bcast(s_av)

        if debug:
            nc.sync.dma_start(out=dbg["d_S"][:], in_=S_all)
            nc.sync.dma_start(out=dbg["d_M"][:], in_=M_all)

        # ============ S8: pass 2 + S@V ============
        out_T, out_T_free = tc.tile([128, HPC, NT], f32, name="out_T")
        oa_parts = pcols(8)
        with tc.tile_pool(name="p2_psum", bufs=2, space="PSUM") as p2_psum, \
             tc.tile_pool(name="sv_psum", bufs=2, space="PSUM") as sv_psum, \
             tc.tile_pool(name="pint", bufs=3) as pint_pool, \
             tc.tile_pool(name="ps_scr", bufs=3) as ps_scr, \
             tc.tile_pool(name="cpool", bufs=2) as cpool:
            for bp_ in range(4):
                b_, h_ = bp_ // 2, bp_ % 2
                tb0 = b_ * TB
                cmv = cpool.tile([3, TB], bf16, name="cmv")
                nc.sync.dma_start(out=cmv[:], in_=rt_bf[:, bp_, :])
                for ih in range(2):
                    isl = slice(tb0 + ih * 1024, tb0 + (ih + 1) * 1024)
                    csl = slice(ih * 1024, (ih + 1) * 1024)
                    sv = sv_psum.tile([128, 1024], f32, name="sv")
                    pending = None
                    for jt in range(16):
                        ps2 = p2_psum.tile([128, 1024], f32, name="ps2")
                        ktile = k_int[:, h_, tb0 + jt * 128 : tb0 + (jt + 1) * 128]
                        qsl = q_int[:, h_, isl]
                        for hf in range(2):
                            sl = slice(hf * 512, (hf + 1) * 512)
                            nc.tensor.matmul(
                                ps2[:, sl], ktile, qsl[:, sl],
                                start=True, stop=False,
                            )
                        for hf in range(2):
                            sl = slice(hf * 512, (hf + 1) * 512)
                            nc.tensor.matmul(
                                ps2[:, sl], ones3[:], cmv[:, csl][:, sl],
                                start=False, stop=True, skip_group_check=True,
                            )
                        PS = ps_scr.tile([128, 1024], f32, name="PS")
                        nc.scalar.activation(
                            PS[:], ps2[:], AF.Exp,
                            scale=s_qk_b[:, 0:1], bias=eb_b[:, 0:1],
                        )
                        pi = pint_pool.tile([128, 1024], bf16, name="pi")
                        rnd(pi[:], PS[:])
                        if pending is not None:
                            for args, kw in pending:
                                nc.tensor.matmul(*args, **kw)
                        vtile = v_int[:, b_ * 16 + jt, h_ * 128 : (h_ + 1) * 128]
                        pending = [
                            (
                                (sv[:, hf * 512 : (hf + 1) * 512], vtile,
                                 pi[:, hf * 512 : (hf + 1) * 512]),
                                dict(start=(jt == 0), stop=(jt == 15)),
                            )
                            for hf in range(2)
                        ]
                    for args, kw in pending:
                        nc.tensor.matmul(*args, **kw)
                    col = bp_ * 2 + ih
                    nc.vector.tensor_scalar(
                        out_T[:, h_, isl], sv[:], s_av_b[:, 0:1], None, op0=OP.mult
                    )
                    nc.vector.tensor_reduce(
                        oa_parts[:, col : col + 1], out_T[:, h_, isl], AX.X, OP.max,
                        apply_absolute_value=True,
                    )

        # ============ S9: out amax -> AR4 -> quantize -> A2A ============
        oc_ = pcols(1)
        nc.vector.tensor_reduce(oc_, oa_parts, AX.X, OP.max)
        ocg = p_reduce_max(oc_)
        nc.sync.dma_start(out=ar4_i[:], in_=ocg[0:1, :])
        nc.gpsimd.collective_compute(
            "AllReduce", OP.max, replica_groups=allg,
            ins=[ar4_i[:].opt()], outs=[ar4_o[:].opt()],
        )
        gO = cols(1)[0:1, :]
        nc.sync.dma_start(out=gO, in_=ar4_o[:])
        s_out = cols(1)[0:1, :]
        nc.vector.tensor_scalar(s_out, gO, 1.0 / 127.0, 1e-8, op0=OP.mult, op1=OP.max)
        i_out = cols(1)[0:1, :]
        nc.vector.reciprocal(i_out, s_out)
        io_b = bcast(i_out)
        s_op = cols(1)[0:1, :]
        nc.vector.tensor_mul(s_op, s_out, s5[:, 4:5])
        s_op_b = bcast(s_op)

        with tc.tile_pool(name="q9scratch", bufs=2) as q9pool:
            for h_ in range(HPC):
                for b_ in range(2):
                    tsl = slice(b_ * TB, (b_ + 1) * TB)
                    m = q9pool.tile([128, TB], f32, name="om", tag="qs9_f32")
                    nc.scalar.activation(m[:], out_T[:, h_, tsl], AF.Copy, scale=io_b[:, 0:1])
                    oi = q9pool.tile([128, TB], bf16, name="oi", tag="qs9_bf16")
                    rnd(oi[:], m[:])
                    nc.sync.dma_start(
                        out=a2a_i[4 * b_ : 4 * (b_ + 1), h_].rearrange("r p t -> p r t"),
                        in_=oi[:].rearrange("p (r t) -> p r t", t=TLOC),
                    )
                if debug:
                    nc.sync.dma_start(out=dbg["d_outT"][:, h_, :], in_=out_T[:, h_, :])
        nc.gpsimd.collective_compute(
            "AllToAll", OP.bypass, replica_groups=allg,
            ins=[a2a_i[:].opt()], outs=[a2a_o[:].opt()],
        )

        # ============ S10: output projection (token-parallel) ============
        out_T_free()
        out_r = out_ext.rearrange("(ot p) t -> p ot t", p=128)
        with tc.tile_pool(name="ogp", bufs=1) as og_pool, \
             tc.tile_pool(name="p7_psum", bufs=4, space="PSUM") as p7_psum, \
             tc.tile_pool(name="wcol7", bufs=3) as wcol_pool7, \
             tc.tile_pool(name="fin", bufs=3) as fin_pool:
            og = og_pool.tile([128, 16, TLOC], bf16, name="og")
            nc.sync.dma_start(out=og[:], in_=a2a_o.rearrange("s h p t -> p (s h) t"))
            for ot in range(16):
                wcol = wcol_pool7.tile([128, 16, 128], bf16, name="wcol")
                nc.sync.dma_start(out=wcol[:], in_=wpg[ot // 2, ot % 2])
                ps = p7_psum.tile([128, TLOC], f32, name="ps_p")
                for ht in range(16):
                    nc.tensor.matmul(
                        ps[:], wcol[:, ht, :], og[:, ht, :],
                        start=(ht == 0), stop=(ht == 15),
                    )
                fin = fin_pool.tile([128, TLOC], f32, name="fin")
                nc.scalar.activation(
                    fin[:], ps[:], AF.Identity,
                    scale=s_op_b[:, 0:1], bias=bp_sb[:, ot : ot + 1],
                )
                nc.sync.dma_start(out=out_r[:, ot, :], in_=fin[:])

        v_int_free()
        k_int_free()
        q_int_free()

        if debug:
            sct = cols(16)[0:1, :]
            nc.vector.tensor_copy(sct[:, 0:5], s5)
            nc.vector.tensor_copy(sct[:, 5:6], s_q)
            nc.vector.tensor_copy(sct[:, 6:8], s_kv)
            nc.vector.tensor_copy(sct[:, 8:9], s_attn)
            nc.vector.tensor_copy(sct[:, 9:10], s_out)
            nc.sync.dma_start(out=dbg["d_scales"][:], in_=sct)

    nc.compile()
    return nc


def _get_compiled(debug=False):
    if debug not in _COMPILED:
        _COMPILED[debug] = _build(debug)
    return _COMPILED[debug]


def make_in_maps(hidden_states, Wq, bq, Wk, bk, Wv, bv, Wp, bp):
    hs = np.asarray(hidden_states, dtype=np.float32)
    wT = [
        np.ascontiguousarray(np.asarray(W, np.float32).T)
        for W in (Wq, Wk, Wv, Wp)
    ]
    bp_t = np.ascontiguousarray(np.asarray(bp, np.float32).reshape(16, 128).T)
    in_maps = []
    for c in range(NC):
        b = c // GROUP
        g = c % GROUP
        osl = slice(c * OS, (c + 1) * OS)
        x_Tc = np.ascontiguousarray(hs[b, g * TLOC : (g + 1) * TLOC, :].T)
        wqkv = np.ascontiguousarray(
            np.stack([wT[w][:, osl] for w in range(3)], axis=0)
        )
        wp_slc = np.ascontiguousarray(wT[3][:, c * OSL : (c + 1) * OSL])
        bqk_c = np.ascontiguousarray(
            np.stack(
                [
                    np.asarray(bq, np.float32)[osl].reshape(HPC, 128).T,
                    np.asarray(bk, np.float32)[osl].reshape(HPC, 128).T,
                ],
                axis=0,
            )
        )
        bv_bc = np.ascontiguousarray(
            np.broadcast_to(np.asarray(bv, np.float32)[None, osl], (128, OS))
        )
        in_maps.append(
            {"x_T": x_Tc, "wqkv": wqkv, "wp_sl": wp_slc, "bqk": bqk_c,
             "bv_b": bv_bc, "bp_t": bp_t}
        )
    return in_maps


def kernel(hidden_states, Wq, bq, Wk, bk, Wv, bv, Wp, bp):
    from concourse.bass_utils import run_bass_kernel_spmd

    debug = bool(int(os.environ.get("KERNEL_DEBUG", "0")))
    trace = bool(int(os.environ.get("KERNEL_TRACE", "0")))
    nc = _get_compiled(debug=debug)
    in_maps = make_in_maps(hidden_states, Wq, bq, Wk, bk, Wv, bv, Wp, bp)
    res = run_bass_kernel_spmd(nc, in_maps, core_ids=list(range(NC)), trace=trace)
    kernel.last_exec_time_ns = res.exec_time_ns
    kernel.last_results = res.results
    kernel.last_res = res

    out = np.empty((B, S, H), dtype=np.float32)
    for c in range(NC):
        b = c // GROUP
        g = c % GROUP
        out[b, g * TLOC : (g + 1) * TLOC, :] = res.results[c]["out"].T
    return out


kernel.last_exec_time_ns = None
kernel.last_results = None
kernel.last_res = None
